# revision 10
# baseline (speedup 1.0000x reference)
"""Trainium2 Bass kernel for nn_MixAttention (GAT-style mixed attention).

Strategy (8 cores, i-sharded over query rows, transposed compute):
  - Device computes scores in transposed layout [j on partitions, i free] so
    out^T += hc_chunk.T @ P^T_chunk contracts over partitions, no transposes.
  - Host passes h_context.T / h_structure.T (layout prep) and param-folded
    projection vectors; real FLOPs (projections, softmax, scores, attention)
    stay on device.
  - Mask passed as complement-uint8; fused on DVE as (-L*maskC + bcB) so exp
    underflows masked entries to exactly 0 (same math as -9e15 additive).
  - exp(alpha - M0) with device-computed bound M0; cancels in the division.
  - rowsum via M=1 ones-stationary matmul sharing the P^T moving operand.
"""

import numpy as np

N = 8192
K = 256
F = 128
NC = 8
S = N // NC  # 1024 rows per core
NEG_L = 1.0e6

_BUILD_CACHE = {}


def _build_program(cA, cB):
    import contextlib

    import concourse.bacc as bacc
    import concourse.tile as tile
    from concourse import mybir

    nc = bacc.Bacc("TRN2", target_bir_lowering=False, debug=False, num_devices=NC)
    dt = mybir.dt
    AF = mybir.ActivationFunctionType
    OP = mybir.AluOpType
    AX = mybir.AxisListType

    hctxT = nc.dram_tensor("hctxT", [K, N], dt.float32, kind="ExternalInput")
    hstrT = nc.dram_tensor("hstrT", [K, N], dt.float32, kind="ExternalInput")
    hctxT_my = nc.dram_tensor("hctxT_my", [K, S], dt.float32, kind="ExternalInput")
    hstrT_my = nc.dram_tensor("hstrT_my", [K, S], dt.float32, kind="ExternalInput")
    wcT = nc.dram_tensor("wcT", [K, F], dt.float32, kind="ExternalInput")
    # vA cols: [srcA proj, dstA proj] (wA folded)
    vA = nc.dram_tensor("vA", [K, 2], dt.float32, kind="ExternalInput")
    # uB cols: [ones (sigma), dstB proj, srcB proj] (wB folded)
    uB = nc.dram_tensor("uB", [K, 3], dt.float32, kind="ExternalInput")
    maskT = nc.dram_tensor("maskT", [N, S], dt.uint8, kind="ExternalInput")
    outT = nc.dram_tensor("outT", [F, S], dt.float32, kind="ExternalOutput")

    NCH = N // 128  # 64 j-chunks
    KC = K // 128   # 2 contraction chunks

    with tile.TileContext(nc) as tc:
        with contextlib.ExitStack() as ctx:
            vecs = ctx.enter_context(tc.tile_pool(name="vecs", bufs=1))
            hcpool = ctx.enter_context(tc.tile_pool(name="hc", bufs=1))

            vA_sb = [vecs.tile([128, 2], dt.float32, name=f"vA{k}") for k in range(KC)]
            uB_sb = [vecs.tile([128, 3], dt.float32, name=f"uB{k}") for k in range(KC)]
            wcT_sb = [vecs.tile([128, F], dt.float32, name=f"wcT{k}") for k in range(KC)]
            for k in range(KC):
                ks = slice(128 * k, 128 * (k + 1))
                nc.sync.dma_start(vA_sb[k][:], vA.ap()[ks, :])
                nc.sync.dma_start(uB_sb[k][:], uB.ap()[ks, :])
                nc.sync.dma_start(wcT_sb[k][:], wcT.ap()[ks, :])

            sgrid = vecs.tile([128, NCH], dt.float32, name="sgrid")
            b2grid = vecs.tile([128, NCH], dt.float32, name="b2grid")
            agrid = vecs.tile([128, NCH], dt.float32, name="agrid")
            a1grid = vecs.tile([128, NCH], dt.float32, name="a1grid")
            sigrow = vecs.tile([1, S], dt.float32, name="sigrow")
            srcBraw = vecs.tile([1, S], dt.float32, name="srcBraw")
            srcArow = vecs.tile([1, S], dt.float32, name="srcArow")

            # ================= phase 0a: structure side =================
            with tc.tile_pool(name="strp", bufs=1) as strp, \
                 tc.tile_pool(name="ps0a", bufs=2, space="PSUM") as ps0a:
                hstrT_sb = [strp.tile([128, N], dt.float32, name=f"hstrT{k}")
                            for k in range(KC)]
                my_str = [strp.tile([128, S], dt.float32, name=f"mystr{k}")
                          for k in range(KC)]
                for k in range(KC):
                    ks = slice(128 * k, 128 * (k + 1))
                    nc.sync.dma_start(hstrT_sb[k][:], hstrT.ap()[ks, :])
                    nc.sync.dma_start(my_str[k][:], hstrT_my.ap()[ks, :])
                for k in range(KC):
                    nc.scalar.activation(hstrT_sb[k][:], hstrT_sb[k][:], AF.Exp)
                    nc.scalar.activation(my_str[k][:], my_str[k][:], AF.Exp)
                # grids: sigma + raw dstB per j-chunk
                for c in range(NCH):
                    cs = slice(128 * c, 128 * (c + 1))
                    psb = ps0a.tile([128, 2], dt.float32, name="psb")
                    for k in range(KC):
                        nc.tensor.matmul(psb[:], hstrT_sb[k][:, cs],
                                         uB_sb[k][:, 0:2],
                                         start=(k == 0), stop=(k == KC - 1))
                    nc.vector.tensor_copy(sgrid[:, c:c + 1], psb[:, 0:1])
                    nc.vector.tensor_copy(b2grid[:, c:c + 1], psb[:, 1:2])
                # rows for my i-slice: sigma and rawSrcB as M=1 matmuls
                psr0 = ps0a.tile([1, S], dt.float32, name="psr0", bufs=1)
                psr1 = ps0a.tile([1, S], dt.float32, name="psr1", bufs=1)
                for k in range(KC):
                    for h in range(S // 512):
                        hs_ = slice(512 * h, 512 * (h + 1))
                        nc.tensor.matmul(psr0[:, hs_], uB_sb[k][:, 0:1],
                                         my_str[k][:, hs_],
                                         start=(k == 0), stop=(k == KC - 1))
                        nc.tensor.matmul(psr1[:, hs_], uB_sb[k][:, 2:3],
                                         my_str[k][:, hs_],
                                         start=(k == 0), stop=(k == KC - 1))
                nc.vector.tensor_copy(sigrow[:], psr0[:])
                nc.vector.tensor_copy(srcBraw[:], psr1[:])

            # ================= phase 0b: context side =================
            hc_sb = [hcpool.tile([128, F], dt.float32, name=f"hc{c}")
                     for c in range(NCH)]
            with tc.tile_pool(name="ctxp", bufs=1) as ctxp, \
                 tc.tile_pool(name="ps0b", bufs=2, space="PSUM") as ps0b:
                hctxT_sb = [ctxp.tile([128, N], dt.float32, name=f"hctxT{k}")
                            for k in range(KC)]
                my_ctx = [ctxp.tile([128, S], dt.float32, name=f"myctx{k}")
                          for k in range(KC)]
                for k in range(KC):
                    ks = slice(128 * k, 128 * (k + 1))
                    nc.sync.dma_start(hctxT_sb[k][:], hctxT.ap()[ks, :])
                    nc.sync.dma_start(my_ctx[k][:], hctxT_my.ap()[ks, :])
                for c in range(NCH):
                    cs = slice(128 * c, 128 * (c + 1))
                    psA = ps0b.tile([128, F], dt.float32, name="psA")
                    psa = ps0b.tile([128, 2], dt.float32, name="psa")
                    for k in range(KC):
                        st, sp = (k == 0), (k == KC - 1)
                        nc.tensor.matmul(psA[:], hctxT_sb[k][:, cs], wcT_sb[k][:],
                                         start=st, stop=sp)
                        nc.tensor.matmul(psa[:], hctxT_sb[k][:, cs], vA_sb[k][:],
                                         start=st, stop=sp)
                    nc.vector.tensor_copy(hc_sb[c][:], psA[:])
                    nc.vector.tensor_copy(a1grid[:, c:c + 1], psa[:, 0:1])
                    if cA != 0.0:
                        nc.vector.tensor_scalar_add(agrid[:, c:c + 1], psa[:, 1:2], cA)
                    else:
                        nc.vector.tensor_copy(agrid[:, c:c + 1], psa[:, 1:2])
                psra = ps0b.tile([1, S], dt.float32, name="psra", bufs=1)
                for k in range(KC):
                    for h in range(S // 512):
                        hs_ = slice(512 * h, 512 * (h + 1))
                        nc.tensor.matmul(psra[:, hs_], vA_sb[k][:, 0:1],
                                         my_ctx[k][:, hs_],
                                         start=(k == 0), stop=(k == KC - 1))
                nc.vector.tensor_copy(srcArow[:], psra[:])

            # ================= phase 0c: derived vectors =================
            bgrid = vecs.tile([128, NCH], dt.float32, name="bgrid")
            srec = vecs.tile([128, NCH], dt.float32, name="srec")
            nc.vector.reciprocal(srec[:], sgrid[:])
            nc.vector.tensor_tensor(bgrid[:], b2grid[:], srec[:], OP.mult)
            if cB != 0.0:
                nc.vector.tensor_scalar_add(bgrid[:], bgrid[:], cB)

            srecrow = vecs.tile([1, S], dt.float32, name="srecrow")
            srcBrow = vecs.tile([1, S], dt.float32, name="srcBrow")
            nc.vector.reciprocal(srecrow[:], sigrow[:])
            nc.vector.tensor_tensor(srcBrow[:], srcBraw[:], srecrow[:], OP.mult)

            ones_row = vecs.tile([1, 128], dt.float32, name="ones_row")
            nc.vector.memset(ones_row[:], 1.0)
            ones_col = vecs.tile([128, 1], dt.float32, name="ones_col")
            nc.vector.memset(ones_col[:], 1.0)

            bcA = vecs.tile([128, S], dt.float32, name="bcA")
            bcB = vecs.tile([128, S], dt.float32, name="bcB")
            with tc.tile_pool(name="ps0c", bufs=1, space="PSUM") as ps0c:
                psbc = ps0c.tile([128, S], dt.float32, name="psbc")
                psbc2 = ps0c.tile([128, S], dt.float32, name="psbc2")
                for h in range(S // 512):
                    hs_ = slice(512 * h, 512 * (h + 1))
                    nc.tensor.matmul(psbc[:, hs_], ones_row[:], srcArow[:, hs_],
                                     start=True, stop=True)
                    nc.tensor.matmul(psbc2[:, hs_], ones_row[:], srcBrow[:, hs_],
                                     start=True, stop=True)
                nc.vector.tensor_copy(bcA[:], psbc[:])
                nc.vector.tensor_copy(bcB[:], psbc2[:])

            # M0 = lrelu(max srcA + max dstA) + lrelu(max srcB + max dstB)
            m = vecs.tile([128, 4], dt.float32, name="mtmp")
            nc.vector.tensor_reduce(m[:, 0:1], agrid[:], AX.X, OP.max)
            nc.vector.tensor_reduce(m[:, 1:2], bgrid[:], AX.X, OP.max)
            nc.vector.tensor_reduce(m[0:1, 2:3], srcArow[:], AX.X, OP.max)
            nc.vector.tensor_reduce(m[0:1, 3:4], srcBrow[:], AX.X, OP.max)
            mg = vecs.tile([1, 2], dt.float32, name="mg")
            nc.gpsimd.tensor_reduce(mg[:, 0:1], m[:, 0:1], AX.C, OP.max)
            nc.gpsimd.tensor_reduce(mg[:, 1:2], m[:, 1:2], AX.C, OP.max)
            sm = vecs.tile([1, 2], dt.float32, name="sm")
            nc.vector.tensor_tensor(sm[:, 0:1], mg[:, 0:1], m[0:1, 2:3], OP.add)
            nc.vector.tensor_tensor(sm[:, 1:2], mg[:, 1:2], m[0:1, 3:4], OP.add)
            lr = vecs.tile([1, 2], dt.float32, name="lr")
            nc.vector.scalar_tensor_tensor(lr[:], sm[:], 0.01, sm[:], OP.mult, OP.max)
            m0 = vecs.tile([1, 1], dt.float32, name="m0")
            nc.vector.tensor_tensor(m0[:], lr[:, 0:1], lr[:, 1:2], OP.add)
            negm0 = vecs.tile([128, 1], dt.float32, name="negm0")
            nc.gpsimd.partition_broadcast(negm0[:], m0[:])
            nc.vector.tensor_scalar_mul(negm0[:], negm0[:], -1.0)

            # ================= phase 1: masked softmax numerator =================
            with tc.tile_pool(name="work", bufs=2) as work, \
                 tc.tile_pool(name="ps1", bufs=1, space="PSUM") as ps1:
                outT_ps = ps1.tile([F, S], dt.float32, name="outT")
                rs_ps = ps1.tile([1, S], dt.float32, name="rs")
                for c in range(NCH):
                    cs = slice(128 * c, 128 * (c + 1))
                    mk = work.tile([128, S], dt.uint8, name="mk")
                    nc.sync.dma_start(mk[:], maskT.ap()[cs, :])
                    u = work.tile([128, S], dt.float32, name="u")
                    nc.vector.scalar_tensor_tensor(
                        u[:], mk[:], -NEG_L, bcB[:], OP.mult, OP.add)
                    tB = work.tile([128, S], dt.float32, name="tB")
                    nc.scalar.activation(tB[:], u[:], AF.Lrelu,
                                         bias=bgrid[:, c:c + 1], scale=1.0,
                                         alpha=0.01)
                    tA = work.tile([128, S], dt.float32, name="tA")
                    nc.scalar.activation(tA[:], bcA[:], AF.Lrelu,
                                         bias=agrid[:, c:c + 1], scale=1.0,
                                         alpha=0.01)
                    s = work.tile([128, S], dt.float32, name="s")
                    nc.vector.tensor_tensor(s[:], tA[:], tB[:], OP.add)
                    P = work.tile([128, S], dt.float32, name="P")
                    nc.scalar.activation(P[:], s[:], AF.Exp, bias=negm0[:],
                                         scale=1.0)
                    st, sp = (c == 0), (c == NCH - 1)
                    for h in range(S // 512):
                        hs_ = slice(512 * h, 512 * (h + 1))
                        nc.tensor.matmul(outT_ps[:, hs_], hc_sb[c][:], P[:, hs_],
                                         start=st, stop=sp)
                        nc.tensor.matmul(rs_ps[:, hs_], ones_col[:], P[:, hs_],
                                         start=st, stop=sp)

                # normalize and write out
                rs_sb = vecs.tile([1, S], dt.float32, name="rs_sb")
                nc.vector.tensor_scalar_add(rs_sb[:], rs_ps[:], 1e-30)
                rrec = vecs.tile([1, S], dt.float32, name="rrec")
                nc.vector.reciprocal(rrec[:], rs_sb[:])
                rbc_ps = ps1.tile([128, S], dt.float32, name="rbc")
                for h in range(S // 512):
                    hs_ = slice(512 * h, 512 * (h + 1))
                    nc.tensor.matmul(rbc_ps[:, hs_], ones_row[:], rrec[:, hs_],
                                     start=True, stop=True)
                rbc = vecs.tile([128, S], dt.float32, name="rbcs")
                nc.vector.tensor_copy(rbc[:], rbc_ps[:])
                out_sb = vecs.tile([F, S], dt.float32, name="out_sb")
                nc.vector.tensor_tensor(out_sb[:], outT_ps[:], rbc[:], OP.mult)
                nc.sync.dma_start(outT.ap(), out_sb[:])

    nc.compile()
    return nc


def kernel(h_context, h_structure, edge_index, Wc_w, Wc_b, Ws_w, Ws_b,
           ac_w, as_w, Ws_coff, Wc_coff):
    from concourse.bass_utils import run_bass_kernel_spmd

    h_context = np.asarray(h_context, np.float32)
    h_structure = np.asarray(h_structure, np.float32)
    Wc_w = np.asarray(Wc_w, np.float32)
    Wc_b = np.asarray(Wc_b, np.float32)
    Ws_w = np.asarray(Ws_w, np.float32)
    Ws_b = np.asarray(Ws_b, np.float32)
    ac_w = np.asarray(ac_w, np.float32)
    as_w = np.asarray(as_w, np.float32)
    ei = np.asarray(edge_index)

    wA = float(abs(np.float32(np.asarray(Ws_coff)[0, 0])))  # scales alpha_c
    wB = float(abs(np.float32(np.asarray(Wc_coff)[0, 0])))  # scales alpha_s

    # Param folding (host, O(K*F) only)
    vA_np = np.stack([Wc_w.T @ ac_w[0, :F], Wc_w.T @ ac_w[0, F:]], axis=1) * wA
    uB_np = np.stack([
        np.ones(K, np.float32),
        wB * (Ws_w.T @ as_w[0, F:]),   # dstB proj
        wB * (Ws_w.T @ as_w[0, :F]),   # srcB proj
    ], axis=1).astype(np.float32)
    # constant (bias) terms: total score adds (src_c + dst_c) per side
    cA = wA * float(Wc_b @ ac_w[0, :F] + Wc_b @ ac_w[0, F:])
    cB = wB * float(Ws_b @ as_w[0, :F] + Ws_b @ as_w[0, F:])

    key = (round(cA, 12), round(cB, 12))
    if key not in _BUILD_CACHE:
        _BUILD_CACHE[key] = _build_program(cA, cB)
    nc = _BUILD_CACHE[key]

    # complement adjacency, transposed: maskCT[j, i] = 0 iff edge i->j
    adjT = np.zeros((N, N), np.uint8)
    adjT[ei[1], ei[0]] = 1
    maskCT = 1 - adjT

    hctxT = np.ascontiguousarray(h_context.T)
    hstrT = np.ascontiguousarray(h_structure.T)
    wcT_np = np.ascontiguousarray(Wc_w.T)
    vA_np = np.ascontiguousarray(vA_np.astype(np.float32))

    in_maps = []
    for d in range(NC):
        sl = slice(S * d, S * (d + 1))
        in_maps.append({
            "hctxT": hctxT,
            "hstrT": hstrT,
            "hctxT_my": np.ascontiguousarray(hctxT[:, sl]),
            "hstrT_my": np.ascontiguousarray(hstrT[:, sl]),
            "wcT": wcT_np,
            "vA": vA_np,
            "uB": np.ascontiguousarray(uB_np),
            "maskT": np.ascontiguousarray(maskCT[:, sl]),
        })

    res = run_bass_kernel_spmd(nc, in_maps, core_ids=list(range(NC)))
    out = np.empty((N, F), np.float32)
    for d in range(NC):
        out[S * d:S * (d + 1), :] = res.results[d]["outT"].T

    # rows with no edges: reference gives uniform attention = mean of hc
    row_deg = np.zeros(N, np.int64)
    np.add.at(row_deg, ei[0], 1)
    empty = row_deg == 0
    if empty.any():
        hc_host = h_context @ Wc_w.T + Wc_b
        out[empty, :] = hc_host.mean(axis=0)

    return out
